# revision 7
# baseline (speedup 1.0000x reference)
"""Attention-gate kernel for Trainium2 (8 NeuronCores, batch-parallel).

Computation (per reference):
    g   = BN_g(input_g @ Wg + bg)          # 1x1 conv 256->128 + BN
    x   = BN_x(input_x @ Wx + bx)          # 1x1 conv 256->128 + BN
    s   = relu(g + x)
    psi = sigmoid(BN_p(s @ Wp + bp))       # 128->1 conv + BN + sigmoid
    out = x * psi                          # [8,128,128,128] f32

Strategy:
  - Shard batch across the 8 cores (core b <- batch b).
  - Host folds BN+bias into the conv weights (W' = W * A, per-output-channel
    scale) and per-channel bias vectors, and pre-transposes the activations
    to channels-first [256, 16384] so the device sees pure contiguous DMA +
    channel-contraction matmuls.
  - Device (per core, 32 blocks x 512 pixels):
      P_g = Wg'^T Xg       (2 matmuls, float32r - fast path; error is
                            attenuated through the sigmoid gate)
      P_x = Wx'^T Xx       (2 matmuls, float32  - exact, direct output factor)
      u   = P_g + P_x                        (DVE)
      s   = relu(u + D_s)                    (ACT, -> float32r)
      P_p = wpo^T s        (1 matmul, float32r; wpo = Wp*A_p replicated to
                            128 columns => result broadcast to all partitions)
      psi = sigmoid(P_p + d_p)               (ACT)
      xv  = P_x + D_x                        (DVE)
      o   = xv * psi                         (DVE)
  - Output is produced channels-first [128, 16384] per core; host transposes
    back. All DMA is >= 2KB-contiguous per partition row.
"""
import numpy as np

B, H, W, CIN, F = 8, 128, 128, 256, 128
NPIX = H * W          # 16384 pixels per batch/core
NBLK = 512            # pixels per block (one PSUM bank at f32)
NB = NPIX // NBLK     # 32 blocks
NCORES = 8
EPS = 1e-3

_CACHE = {}


def _build_program():
    import sys
    if "/opt/trn_rl_repo" not in sys.path:
        sys.path.insert(0, "/opt/trn_rl_repo")
    from contextlib import ExitStack
    import concourse.bacc as bacc
    import concourse.tile as tile
    from concourse import mybir

    f32 = mybir.dt.float32
    f32r = mybir.dt.float32r
    AF = mybir.ActivationFunctionType

    nc = bacc.Bacc("TRN2", target_bir_lowering=False, debug=False,
                   enable_asserts=False)
    xg = nc.dram_tensor("xg", [CIN, NPIX], f32r, kind="ExternalInput").ap()
    xx = nc.dram_tensor("xx", [CIN, NPIX], f32, kind="ExternalInput").ap()
    wg = nc.dram_tensor("wg", [CIN, F], f32r, kind="ExternalInput").ap()
    wx = nc.dram_tensor("wx", [CIN, F], f32, kind="ExternalInput").ap()
    wpo = nc.dram_tensor("wpo", [F, 128], f32r, kind="ExternalInput").ap()
    dvec = nc.dram_tensor("dvec", [128, 3], f32, kind="ExternalInput").ap()
    out_t = nc.dram_tensor("out_t", [F, NPIX], f32, kind="ExternalOutput").ap()

    with tile.TileContext(nc) as tc:
        with ExitStack() as ctx:
            consts = ctx.enter_context(tc.tile_pool(name="consts", bufs=1))
            io_in = ctx.enter_context(tc.tile_pool(name="io_in", bufs=3))
            work = ctx.enter_context(tc.tile_pool(name="work", bufs=4))
            io_out = ctx.enter_context(tc.tile_pool(name="io_out", bufs=3))
            px_pool = ctx.enter_context(tc.tile_pool(name="px", bufs=3, space="PSUM"))
            pp_pool = ctx.enter_context(tc.tile_pool(name="pp", bufs=3, space="PSUM"))

            wg0 = consts.tile([128, F], f32r)
            nc.sync.dma_start(out=wg0, in_=wg[0:128, :])
            wg1 = consts.tile([128, F], f32r)
            nc.sync.dma_start(out=wg1, in_=wg[128:256, :])
            wx0 = consts.tile([128, F], f32)
            nc.sync.dma_start(out=wx0, in_=wx[0:128, :])
            wx1 = consts.tile([128, F], f32)
            nc.sync.dma_start(out=wx1, in_=wx[128:256, :])
            wpo_sb = consts.tile([F, 128], f32r)
            nc.sync.dma_start(out=wpo_sb, in_=wpo)
            dv = consts.tile([128, 3], f32)
            nc.sync.dma_start(out=dv, in_=dvec)
            d_s = dv[:, 0:1]
            d_x = dv[:, 1:2]
            d_p = dv[:, 2:3]

            GB = 2                # sub-blocks per DMA group
            GBLK = GB * NBLK      # 1024 pixels per DMA (4KB rows)
            for g in range(NB // GB):
                gsl = slice(g * GBLK, (g + 1) * GBLK)
                xx_b0 = io_in.tile([128, GBLK], f32, name=f"xxb0_{g}", tag="xxb0")
                nc.sync.dma_start(out=xx_b0, in_=xx[0:128, gsl])
                xx_b1 = io_in.tile([128, GBLK], f32, name=f"xxb1_{g}", tag="xxb1")
                nc.sync.dma_start(out=xx_b1, in_=xx[128:256, gsl])
                xg_b0 = io_in.tile([128, GBLK], f32r, name=f"xgb0_{g}", tag="xgb0")
                nc.sync.dma_start(out=xg_b0, in_=xg[0:128, gsl])
                xg_b1 = io_in.tile([128, GBLK], f32r, name=f"xgb1_{g}", tag="xgb1")
                nc.sync.dma_start(out=xg_b1, in_=xg[128:256, gsl])

                o_big = io_out.tile([128, GBLK], f32, name=f"ob_{g}", tag="ob")

                for j in range(GB):
                    i = g * GB + j
                    sl = slice(j * NBLK, (j + 1) * NBLK)

                    # P_x = Wx'^T Xx (fp32, exact)
                    ps_x = px_pool.tile([128, NBLK], f32, name=f"ps_x_{i}", tag="ps_x")
                    nc.tensor.matmul(ps_x, wx0, xx_b0[:, sl], start=True, stop=False)
                    nc.tensor.matmul(ps_x, wx1, xx_b1[:, sl], start=False, stop=True)

                    # xv = P_x + D_x -> SBUF (frees the bank value for reuse)
                    xv = work.tile([128, NBLK], f32, name=f"xv_{i}", tag="xv")
                    nc.vector.tensor_scalar_add(xv, ps_x, d_x)

                    # accumulate the g-branch (f32r) onto the same bank:
                    # bank becomes P_s = P_g + P_x (WAR on xv read; Tile serializes)
                    nc.tensor.matmul(ps_x, wg0, xg_b0[:, sl], start=False, stop=False,
                                     skip_group_check=True)
                    nc.tensor.matmul(ps_x, wg1, xg_b1[:, sl], start=False, stop=True,
                                     skip_group_check=True)

                    s_sb = work.tile([128, NBLK], f32r, name=f"s_{i}", tag="s")
                    nc.scalar.activation(s_sb, ps_x, AF.Relu, bias=d_s, scale=1.0)

                    ps_p = pp_pool.tile([128, NBLK], f32, name=f"ps_p_{i}", tag="ps_p")
                    nc.tensor.matmul(ps_p, wpo_sb, s_sb, start=True, stop=True)

                    psi = work.tile([128, NBLK], f32, name=f"psi_{i}", tag="psi")
                    nc.scalar.activation(psi, ps_p, AF.Sigmoid, bias=d_p, scale=1.0)

                    nc.vector.tensor_mul(o_big[:, sl], xv, psi)

                nc.sync.dma_start(out=out_t[:, gsl], in_=o_big)

    nc.compile()
    return nc


def _get_program():
    if "nc" not in _CACHE:
        _CACHE["nc"] = _build_program()
    return _CACHE["nc"]


def kernel(input_g, input_x, Wg, bg, gamma_g, beta_g, mean_g, var_g,
           Wx, bx, gamma_x, beta_x, mean_x, var_x,
           Wp, bp, gamma_p, beta_p, mean_p, var_p):
    import sys
    if "/opt/trn_rl_repo" not in sys.path:
        sys.path.insert(0, "/opt/trn_rl_repo")
    from concourse import bass_utils

    nc = _get_program()

    # Fold BN (+conv bias) into weights/biases on host, in float64 for accuracy.
    f8 = np.float64
    A_g = (gamma_g.astype(f8) / np.sqrt(var_g.astype(f8) + EPS))
    C_g = beta_g.astype(f8) - mean_g.astype(f8) * A_g + bg.astype(f8) * A_g
    A_x = (gamma_x.astype(f8) / np.sqrt(var_x.astype(f8) + EPS))
    C_x = beta_x.astype(f8) - mean_x.astype(f8) * A_x + bx.astype(f8) * A_x
    A_p = (gamma_p.astype(f8) / np.sqrt(var_p.astype(f8) + EPS))[0]
    C_p = (beta_p.astype(f8) - mean_p.astype(f8) * A_p)[0]

    wg_eff = (Wg.astype(f8) * A_g[None, :]).astype(np.float32)
    wx_eff = (Wx.astype(f8) * A_x[None, :]).astype(np.float32)
    wpo = np.ascontiguousarray(
        np.repeat((Wp[:, 0].astype(f8) * A_p)[:, None], 128, axis=1)
    ).astype(np.float32)
    d_s = (C_g + C_x).astype(np.float32)
    d_x = C_x.astype(np.float32)
    d_p = np.full((128,), A_p * bp.astype(f8)[0] + C_p, dtype=np.float32)
    dvec = np.ascontiguousarray(np.stack([d_s, d_x, d_p], axis=1))

    in_maps = []
    for b in range(NCORES):
        xg_t = np.ascontiguousarray(input_g[b].reshape(NPIX, CIN).T)
        xx_t = np.ascontiguousarray(input_x[b].reshape(NPIX, CIN).T)
        in_maps.append(dict(xg=xg_t, xx=xx_t, wg=wg_eff, wx=wx_eff,
                            wpo=wpo, dvec=dvec))

    res = bass_utils.run_bass_kernel_spmd(nc, in_maps,
                                          core_ids=list(range(NCORES)))
    _CACHE["last_results"] = res

    out = np.empty((B, H, W, F), np.float32)
    for b in range(NCORES):
        out[b] = res.results[b]["out_t"].T.reshape(H, W, F)
    return out


# revision 8
# speedup vs baseline: 1.1865x; 1.1865x over previous
"""Attention-gate kernel for Trainium2 (8 NeuronCores, batch-parallel).

Computation (per reference):
    g   = BN_g(input_g @ Wg + bg)          # 1x1 conv 256->128 + BN
    x   = BN_x(input_x @ Wx + bx)          # 1x1 conv 256->128 + BN
    s   = relu(g + x)
    psi = sigmoid(BN_p(s @ Wp + bp))       # 128->1 conv + BN + sigmoid
    out = x * psi                          # [8,128,128,128] f32

Strategy:
  - Shard batch across the 8 cores (core b <- batch b).
  - Host folds BN+bias into the conv weights (W' = W * A, per-output-channel
    scale) and per-channel bias vectors, and pre-transposes the activations
    to channels-first [256, 16384] so the device sees pure contiguous DMA +
    channel-contraction matmuls.
  - Device (per core, 32 blocks x 512 pixels):
      P_g = Wg'^T Xg       (2 matmuls, float32r - fast path; error is
                            attenuated through the sigmoid gate)
      P_x = Wx'^T Xx       (2 matmuls, float32  - exact, direct output factor)
      u   = P_g + P_x                        (DVE)
      s   = relu(u + D_s)                    (ACT, -> float32r)
      P_p = wpo^T s        (1 matmul, float32r; wpo = Wp*A_p replicated to
                            128 columns => result broadcast to all partitions)
      psi = sigmoid(P_p + d_p)               (ACT)
      xv  = P_x + D_x                        (DVE)
      o   = xv * psi                         (DVE)
  - Output is produced channels-first [128, 16384] per core; host transposes
    back. All DMA is >= 2KB-contiguous per partition row.
"""
import numpy as np

B, H, W, CIN, F = 8, 128, 128, 256, 128
NPIX = H * W          # 16384 pixels per batch/core
NBLK = 512            # pixels per block (one PSUM bank at f32)
NB = NPIX // NBLK     # 32 blocks
NCORES = 8
EPS = 1e-3

_CACHE = {}


def _build_program():
    import sys
    if "/opt/trn_rl_repo" not in sys.path:
        sys.path.insert(0, "/opt/trn_rl_repo")
    from contextlib import ExitStack
    import concourse.bacc as bacc
    import concourse.tile as tile
    from concourse import mybir

    f32 = mybir.dt.float32
    f32r = mybir.dt.float32r
    AF = mybir.ActivationFunctionType

    nc = bacc.Bacc("TRN2", target_bir_lowering=False, debug=False,
                   enable_asserts=False)
    xg = nc.dram_tensor("xg", [CIN, NPIX], f32r, kind="ExternalInput").ap()
    xx = nc.dram_tensor("xx", [CIN, NPIX], f32, kind="ExternalInput").ap()
    wg = nc.dram_tensor("wg", [CIN, F], f32r, kind="ExternalInput").ap()
    wx = nc.dram_tensor("wx", [CIN, F], f32, kind="ExternalInput").ap()
    wpo = nc.dram_tensor("wpo", [F, 128], f32r, kind="ExternalInput").ap()
    dvec = nc.dram_tensor("dvec", [128, 3], f32, kind="ExternalInput").ap()
    out_t = nc.dram_tensor("out_t", [F, NPIX], f32, kind="ExternalOutput").ap()

    with tile.TileContext(nc) as tc:
        with ExitStack() as ctx:
            consts = ctx.enter_context(tc.tile_pool(name="consts", bufs=1))
            io_in = ctx.enter_context(tc.tile_pool(name="io_in", bufs=2))
            work = ctx.enter_context(tc.tile_pool(name="work", bufs=3))
            io_out = ctx.enter_context(tc.tile_pool(name="io_out", bufs=2))
            px_pool = ctx.enter_context(tc.tile_pool(name="px", bufs=3, space="PSUM"))
            pp_pool = ctx.enter_context(tc.tile_pool(name="pp", bufs=3, space="PSUM"))

            wg0 = consts.tile([128, F], f32r)
            nc.sync.dma_start(out=wg0, in_=wg[0:128, :])
            wg1 = consts.tile([128, F], f32r)
            nc.sync.dma_start(out=wg1, in_=wg[128:256, :])
            wx0 = consts.tile([128, F], f32)
            nc.sync.dma_start(out=wx0, in_=wx[0:128, :])
            wx1 = consts.tile([128, F], f32)
            nc.sync.dma_start(out=wx1, in_=wx[128:256, :])
            wpo_sb = consts.tile([F, 128], f32r)
            nc.sync.dma_start(out=wpo_sb, in_=wpo)
            dv = consts.tile([128, 3], f32)
            nc.sync.dma_start(out=dv, in_=dvec)
            d_s = dv[:, 0:1]
            d_x = dv[:, 1:2]
            d_p = dv[:, 2:3]

            GB = 4                # sub-blocks per DMA group
            GBLK = GB * NBLK      # 2048 pixels per DMA (8KB rows)
            for g in range(NB // GB):
                gsl = slice(g * GBLK, (g + 1) * GBLK)
                xx_b0 = io_in.tile([128, GBLK], f32, name=f"xxb0_{g}", tag="xxb0")
                nc.sync.dma_start(out=xx_b0, in_=xx[0:128, gsl])
                xx_b1 = io_in.tile([128, GBLK], f32, name=f"xxb1_{g}", tag="xxb1")
                nc.sync.dma_start(out=xx_b1, in_=xx[128:256, gsl])
                xg_b0 = io_in.tile([128, GBLK], f32r, name=f"xgb0_{g}", tag="xgb0")
                nc.sync.dma_start(out=xg_b0, in_=xg[0:128, gsl])
                xg_b1 = io_in.tile([128, GBLK], f32r, name=f"xgb1_{g}", tag="xgb1")
                nc.sync.dma_start(out=xg_b1, in_=xg[128:256, gsl])

                o_big = io_out.tile([128, GBLK], f32, name=f"ob_{g}", tag="ob")

                for j in range(GB):
                    i = g * GB + j
                    sl = slice(j * NBLK, (j + 1) * NBLK)

                    # P_x = Wx'^T Xx (fp32, exact)
                    ps_x = px_pool.tile([128, NBLK], f32, name=f"ps_x_{i}", tag="ps_x")
                    nc.tensor.matmul(ps_x, wx0, xx_b0[:, sl], start=True, stop=False)
                    nc.tensor.matmul(ps_x, wx1, xx_b1[:, sl], start=False, stop=True)

                    # xv = P_x + D_x -> SBUF (frees the bank value for reuse)
                    xv = work.tile([128, NBLK], f32, name=f"xv_{i}", tag="xv")
                    nc.vector.tensor_scalar_add(xv, ps_x, d_x)

                    # accumulate the g-branch (f32r) onto the same bank:
                    # bank becomes P_s = P_g + P_x (WAR on xv read; Tile serializes)
                    nc.tensor.matmul(ps_x, wg0, xg_b0[:, sl], start=False, stop=False,
                                     skip_group_check=True)
                    nc.tensor.matmul(ps_x, wg1, xg_b1[:, sl], start=False, stop=True,
                                     skip_group_check=True)

                    s_sb = work.tile([128, NBLK], f32r, name=f"s_{i}", tag="s")
                    nc.scalar.activation(s_sb, ps_x, AF.Relu, bias=d_s, scale=1.0)

                    ps_p = pp_pool.tile([128, NBLK], f32, name=f"ps_p_{i}", tag="ps_p")
                    nc.tensor.matmul(ps_p, wpo_sb, s_sb, start=True, stop=True)

                    psi = work.tile([128, NBLK], f32, name=f"psi_{i}", tag="psi")
                    nc.scalar.activation(psi, ps_p, AF.Sigmoid, bias=d_p, scale=1.0)

                    nc.vector.tensor_mul(o_big[:, sl], xv, psi)

                nc.sync.dma_start(out=out_t[:, gsl], in_=o_big)

    nc.compile()
    return nc


def _get_program():
    if "nc" not in _CACHE:
        _CACHE["nc"] = _build_program()
    return _CACHE["nc"]


def kernel(input_g, input_x, Wg, bg, gamma_g, beta_g, mean_g, var_g,
           Wx, bx, gamma_x, beta_x, mean_x, var_x,
           Wp, bp, gamma_p, beta_p, mean_p, var_p):
    import sys
    if "/opt/trn_rl_repo" not in sys.path:
        sys.path.insert(0, "/opt/trn_rl_repo")
    from concourse import bass_utils

    nc = _get_program()

    # Fold BN (+conv bias) into weights/biases on host, in float64 for accuracy.
    f8 = np.float64
    A_g = (gamma_g.astype(f8) / np.sqrt(var_g.astype(f8) + EPS))
    C_g = beta_g.astype(f8) - mean_g.astype(f8) * A_g + bg.astype(f8) * A_g
    A_x = (gamma_x.astype(f8) / np.sqrt(var_x.astype(f8) + EPS))
    C_x = beta_x.astype(f8) - mean_x.astype(f8) * A_x + bx.astype(f8) * A_x
    A_p = (gamma_p.astype(f8) / np.sqrt(var_p.astype(f8) + EPS))[0]
    C_p = (beta_p.astype(f8) - mean_p.astype(f8) * A_p)[0]

    wg_eff = (Wg.astype(f8) * A_g[None, :]).astype(np.float32)
    wx_eff = (Wx.astype(f8) * A_x[None, :]).astype(np.float32)
    wpo = np.ascontiguousarray(
        np.repeat((Wp[:, 0].astype(f8) * A_p)[:, None], 128, axis=1)
    ).astype(np.float32)
    d_s = (C_g + C_x).astype(np.float32)
    d_x = C_x.astype(np.float32)
    d_p = np.full((128,), A_p * bp.astype(f8)[0] + C_p, dtype=np.float32)
    dvec = np.ascontiguousarray(np.stack([d_s, d_x, d_p], axis=1))

    in_maps = []
    for b in range(NCORES):
        xg_t = np.ascontiguousarray(input_g[b].reshape(NPIX, CIN).T)
        xx_t = np.ascontiguousarray(input_x[b].reshape(NPIX, CIN).T)
        in_maps.append(dict(xg=xg_t, xx=xx_t, wg=wg_eff, wx=wx_eff,
                            wpo=wpo, dvec=dvec))

    res = bass_utils.run_bass_kernel_spmd(nc, in_maps,
                                          core_ids=list(range(NCORES)))
    _CACHE["last_results"] = res

    out = np.empty((B, H, W, F), np.float32)
    for b in range(NCORES):
        out[b] = res.results[b]["out_t"].T.reshape(H, W, F)
    return out


# revision 32
# speedup vs baseline: 1.3156x; 1.1088x over previous
"""Attention-gate kernel for Trainium2 (8 NeuronCores, batch-parallel).

Computation (per reference):
    g   = BN_g(input_g @ Wg + bg)          # 1x1 conv 256->128 + BN
    x   = BN_x(input_x @ Wx + bx)          # 1x1 conv 256->128 + BN
    s   = relu(g + x)
    psi = sigmoid(BN_p(s @ Wp + bp))       # 128->1 conv + BN + sigmoid
    out = x * psi                          # [8,128,128,128] f32

Strategy (memory-bound target; ~42MB of HBM traffic per core):
  - Shard batch across the 8 cores (core b <- batch b).
  - Host folds BN+bias into the conv weights (W' = W * A, per-output-channel
    scale) and per-channel bias vectors, and pre-transposes the activations
    to channels-first [256, 16384] so the device sees pure contiguous DMA +
    channel-contraction matmuls (no on-device transposes).
  - input_x is uploaded as a double-bf16 pair (hi + lo, same total bytes as
    f32): the x-conv P_x = Xh Wh + Xl Wh + Xh Wl runs at bf16 matmul speed
    with ~1e-5 accuracy (bf16 products are exact in fp32 PSUM accumulate).
    input_g stays float32r (fast PE path); its rounding error only enters
    through the sigmoid gate, which attenuates it.
  - Device (per core, 32 blocks x 512 pixels, DMA-grouped by 4 blocks,
    1-stage software pipeline so the PE never stalls on the shared-bank WAR):
      P_x  = double-bf16 conv            (6 matmuls -> PSUM bank)
      xv   = P_x + D_x                   (DVE -> SBUF)
      P_s  = P_x += Wg'^T Xg             (2 f32r matmuls accumulate onto the
                                          same bank after xv is read)
      s    = relu(P_s + D_s)             (ACT, -> float32r SBUF)
      P_p  = wpo^T s   (1 f32r matmul; wpo = Wp*A_p replicated to 128
                        columns => gate broadcast to all partitions)
      psi  = sigmoid(P_p + d_p)          (ACT)
      o    = xv * psi                    (DVE)
  - Output is produced channels-first [128, 16384] per core; host transposes
    back. All DMA rows are >= 4KB contiguous.
"""
import numpy as np

B, H, W, CIN, F = 8, 128, 128, 256, 128
NPIX = H * W          # 16384 pixels per batch/core
NBLK = 512            # pixels per block (one PSUM bank at f32)
NB = NPIX // NBLK     # 32 blocks
NCORES = 8
EPS = 1e-3

_CACHE = {}


def _build_program(npix=NPIX):
    import sys
    if "/opt/trn_rl_repo" not in sys.path:
        sys.path.insert(0, "/opt/trn_rl_repo")
    from contextlib import ExitStack
    import concourse.bacc as bacc
    import concourse.tile as tile
    from concourse import mybir

    f32 = mybir.dt.float32
    f32r = mybir.dt.float32r
    bf16 = mybir.dt.bfloat16
    AF = mybir.ActivationFunctionType

    nc = bacc.Bacc("TRN2", target_bir_lowering=False, debug=False,
                   enable_asserts=False)
    NPIXL = npix
    xg = nc.dram_tensor("xg", [CIN, NPIXL], f32r, kind="ExternalInput").ap()
    xxh = nc.dram_tensor("xxh", [CIN, NPIXL], bf16, kind="ExternalInput").ap()
    xxl = nc.dram_tensor("xxl", [CIN, NPIXL], bf16, kind="ExternalInput").ap()
    wg = nc.dram_tensor("wg", [CIN, F], f32r, kind="ExternalInput").ap()
    wxh = nc.dram_tensor("wxh", [CIN, F], bf16, kind="ExternalInput").ap()
    wxl = nc.dram_tensor("wxl", [CIN, F], bf16, kind="ExternalInput").ap()
    wpo = nc.dram_tensor("wpo", [F, 128], f32r, kind="ExternalInput").ap()
    dvec = nc.dram_tensor("dvec", [128, 3], f32, kind="ExternalInput").ap()
    out_t = nc.dram_tensor("out_t", [F, NPIXL], f32, kind="ExternalOutput").ap()

    with tile.TileContext(nc) as tc:
        with ExitStack() as ctx:
            consts = ctx.enter_context(tc.tile_pool(name="consts", bufs=1))
            io_in = ctx.enter_context(tc.tile_pool(name="io_in", bufs=3))
            work = ctx.enter_context(tc.tile_pool(name="work", bufs=4))
            io_out = ctx.enter_context(tc.tile_pool(name="io_out", bufs=2))
            px_pool = ctx.enter_context(tc.tile_pool(name="px", bufs=4, space="PSUM"))
            pp_pool = ctx.enter_context(tc.tile_pool(name="pp", bufs=3, space="PSUM"))

            wxh0 = consts.tile([128, F], bf16)
            nc.gpsimd.dma_start(out=wxh0, in_=wxh[0:128, :])
            wxh1 = consts.tile([128, F], bf16)
            nc.gpsimd.dma_start(out=wxh1, in_=wxh[128:256, :])
            wxl0 = consts.tile([128, F], bf16)
            nc.gpsimd.dma_start(out=wxl0, in_=wxl[0:128, :])
            wxl1 = consts.tile([128, F], bf16)
            nc.gpsimd.dma_start(out=wxl1, in_=wxl[128:256, :])
            dv = consts.tile([128, 3], f32)
            nc.gpsimd.dma_start(out=dv, in_=dvec)
            wg0 = consts.tile([128, F], f32r)
            nc.gpsimd.dma_start(out=wg0, in_=wg[0:128, :])
            wg1 = consts.tile([128, F], f32r)
            nc.gpsimd.dma_start(out=wg1, in_=wg[128:256, :])
            wpo_sb = consts.tile([F, 128], f32r)
            nc.gpsimd.dma_start(out=wpo_sb, in_=wpo)
            d_s = dv[:, 0:1]
            d_x = dv[:, 1:2]
            d_p = dv[:, 2:3]

            NBL = NPIXL // NBLK
            GSIZES = [4] * (NBL // 4)
            assert sum(GSIZES) == NBL

            # One-stage software pipeline across sub-blocks: emit sub-block
            # i's x-matmuls + xv, THEN sub-block i-1's gate tail (g-matmuls,
            # relu, psi, sigmoid, final mul).  The PE static order then
            # interleaves block i's x-matmuls into the window where block
            # i-1's g-matmuls wait on its xv read (WAR on the shared bank),
            # keeping the PE dense (HAM stays warm).
            LAG = 1
            pendq = []  # (i, ps_x, xv, o_slice, xg0_sl, xg1_sl, group_done)

            def finish(p):
                i, ps_x, xv, o_sl, xg0_sl, xg1_sl, group_done = p
                nc.tensor.matmul(ps_x, wg0, xg0_sl, start=False, stop=False,
                                 skip_group_check=True)
                nc.tensor.matmul(ps_x, wg1, xg1_sl, start=False, stop=True,
                                 skip_group_check=True)
                s_sb = work.tile([128, NBLK], f32r, name=f"s_{i}", tag="s")
                nc.scalar.activation(s_sb, ps_x, AF.Relu, bias=d_s, scale=1.0)
                ps_p = pp_pool.tile([128, NBLK], f32, name=f"ps_p_{i}", tag="ps_p")
                nc.tensor.matmul(ps_p, wpo_sb, s_sb, start=True, stop=True)
                psi = work.tile([128, NBLK], f32, name=f"psi_{i}", tag="psi")
                nc.scalar.activation(psi, ps_p, AF.Sigmoid, bias=d_p, scale=1.0)
                nc.vector.tensor_mul(o_sl, xv, psi)
                if group_done is not None:
                    group_done()

            gstart = 0
            for g, GB in enumerate(GSIZES):
                GBLK = GB * NBLK
                gsl = slice(gstart * NBLK, gstart * NBLK + GBLK)
                xxh_b0 = io_in.tile([128, GBLK], bf16, name=f"xxhb0_{g}", tag="xxhb0")
                nc.sync.dma_start(out=xxh_b0, in_=xxh[0:128, gsl])
                xxh_b1 = io_in.tile([128, GBLK], bf16, name=f"xxhb1_{g}", tag="xxhb1")
                nc.sync.dma_start(out=xxh_b1, in_=xxh[128:256, gsl])
                xxl_b0 = io_in.tile([128, GBLK], bf16, name=f"xxlb0_{g}", tag="xxlb0")
                nc.sync.dma_start(out=xxl_b0, in_=xxl[0:128, gsl])
                xxl_b1 = io_in.tile([128, GBLK], bf16, name=f"xxlb1_{g}", tag="xxlb1")
                nc.sync.dma_start(out=xxl_b1, in_=xxl[128:256, gsl])
                xg_b0 = io_in.tile([128, GBLK], f32r, name=f"xgb0_{g}", tag="xgb0")
                nc.sync.dma_start(out=xg_b0, in_=xg[0:128, gsl])
                xg_b1 = io_in.tile([128, GBLK], f32r, name=f"xgb1_{g}", tag="xgb1")
                nc.sync.dma_start(out=xg_b1, in_=xg[128:256, gsl])

                o_big = io_out.tile([128, GBLK], f32, name=f"ob_{g}", tag="ob")

                def group_flush(gsl=gsl, o_big=o_big):
                    nc.sync.dma_start(out=out_t[:, gsl], in_=o_big)

                for j in range(GB):
                    i = gstart + j
                    sl = slice(j * NBLK, (j + 1) * NBLK)

                    # P_x = Wx'^T Xx via double-bf16: Xh Wh + Xl Wh + Xh Wl
                    ps_x = px_pool.tile([128, NBLK], f32, name=f"ps_x_{i}", tag="ps_x")
                    nc.tensor.matmul(ps_x, wxh0, xxh_b0[:, sl], start=True, stop=False)
                    nc.tensor.matmul(ps_x, wxh1, xxh_b1[:, sl], start=False, stop=False)
                    nc.tensor.matmul(ps_x, wxh0, xxl_b0[:, sl], start=False, stop=False)
                    nc.tensor.matmul(ps_x, wxh1, xxl_b1[:, sl], start=False, stop=False)
                    nc.tensor.matmul(ps_x, wxl0, xxh_b0[:, sl], start=False, stop=False)
                    nc.tensor.matmul(ps_x, wxl1, xxh_b1[:, sl], start=False, stop=True)

                    # xv = P_x + D_x -> SBUF (frees the bank value for reuse)
                    xv = work.tile([128, NBLK], f32, name=f"xv_{i}", tag="xv")
                    nc.vector.tensor_scalar_add(xv, ps_x, d_x)

                    if len(pendq) >= LAG:
                        finish(pendq.pop(0))
                    pendq.append((i, ps_x, xv, o_big[:, sl],
                                  xg_b0[:, sl], xg_b1[:, sl],
                                  group_flush if j == GB - 1 else None))

                gstart += GB
            for p_ in pendq:
                finish(p_)

    nc.compile()
    return nc


def _get_program():
    if "nc" not in _CACHE:
        _CACHE["nc"] = _build_program()
    return _CACHE["nc"]


def kernel(input_g, input_x, Wg, bg, gamma_g, beta_g, mean_g, var_g,
           Wx, bx, gamma_x, beta_x, mean_x, var_x,
           Wp, bp, gamma_p, beta_p, mean_p, var_p):
    import sys
    if "/opt/trn_rl_repo" not in sys.path:
        sys.path.insert(0, "/opt/trn_rl_repo")
    from concourse import bass_utils

    # Accept jax/np arrays alike.
    (input_g, input_x, Wg, bg, gamma_g, beta_g, mean_g, var_g,
     Wx, bx, gamma_x, beta_x, mean_x, var_x,
     Wp, bp, gamma_p, beta_p, mean_p, var_p) = (
        np.asarray(a) for a in (
            input_g, input_x, Wg, bg, gamma_g, beta_g, mean_g, var_g,
            Wx, bx, gamma_x, beta_x, mean_x, var_x,
            Wp, bp, gamma_p, beta_p, mean_p, var_p))

    nc = _get_program()

    # Fold BN (+conv bias) into weights/biases on host, in float64 for accuracy.
    f8 = np.float64
    A_g = (gamma_g.astype(f8) / np.sqrt(var_g.astype(f8) + EPS))
    C_g = beta_g.astype(f8) - mean_g.astype(f8) * A_g + bg.astype(f8) * A_g
    A_x = (gamma_x.astype(f8) / np.sqrt(var_x.astype(f8) + EPS))
    C_x = beta_x.astype(f8) - mean_x.astype(f8) * A_x + bx.astype(f8) * A_x
    A_p = (gamma_p.astype(f8) / np.sqrt(var_p.astype(f8) + EPS))[0]
    C_p = (beta_p.astype(f8) - mean_p.astype(f8) * A_p)[0]

    import ml_dtypes
    bf16 = ml_dtypes.bfloat16

    wg_eff = (Wg.astype(f8) * A_g[None, :]).astype(np.float32)
    wx_eff = (Wx.astype(f8) * A_x[None, :]).astype(np.float32)
    wx_hi = wx_eff.astype(bf16)
    wx_lo = (wx_eff - wx_hi.astype(np.float32)).astype(bf16)
    wpo = np.ascontiguousarray(
        np.repeat((Wp[:, 0].astype(f8) * A_p)[:, None], 128, axis=1)
    ).astype(np.float32)
    d_s = (C_g + C_x).astype(np.float32)
    d_x = C_x.astype(np.float32)
    d_p = np.full((128,), A_p * bp.astype(f8)[0] + C_p, dtype=np.float32)
    dvec = np.ascontiguousarray(np.stack([d_s, d_x, d_p], axis=1))

    in_maps = []
    for b in range(NCORES):
        xg_t = np.ascontiguousarray(input_g[b].reshape(NPIX, CIN).T)
        xx_t = np.ascontiguousarray(input_x[b].reshape(NPIX, CIN).T)
        xx_hi = xx_t.astype(bf16)
        xx_lo = (xx_t - xx_hi.astype(np.float32)).astype(bf16)
        in_maps.append(dict(xg=xg_t, xxh=xx_hi, xxl=xx_lo,
                            wg=wg_eff, wxh=wx_hi, wxl=wx_lo,
                            wpo=wpo, dvec=dvec))

    res = bass_utils.run_bass_kernel_spmd(nc, in_maps,
                                          core_ids=list(range(NCORES)))
    _CACHE["last_results"] = res

    out = np.empty((B, H, W, F), np.float32)
    for b in range(NCORES):
        out[b] = res.results[b]["out_t"].T.reshape(H, W, F)
    return out


# revision 33
# speedup vs baseline: 1.3356x; 1.0152x over previous
"""Attention-gate kernel for Trainium2 (8 NeuronCores, batch-parallel).

Computation (per reference):
    g   = BN_g(input_g @ Wg + bg)          # 1x1 conv 256->128 + BN
    x   = BN_x(input_x @ Wx + bx)          # 1x1 conv 256->128 + BN
    s   = relu(g + x)
    psi = sigmoid(BN_p(s @ Wp + bp))       # 128->1 conv + BN + sigmoid
    out = x * psi                          # [8,128,128,128] f32

Strategy (memory-bound target; ~42MB of HBM traffic per core):
  - Shard batch across the 8 cores (core b <- batch b).
  - Host folds BN+bias into the conv weights (W' = W * A, per-output-channel
    scale) and per-channel bias vectors, and pre-transposes the activations
    to channels-first [256, 16384] so the device sees pure contiguous DMA +
    channel-contraction matmuls (no on-device transposes).
  - input_x is uploaded as a double-bf16 pair (hi + lo, same total bytes as
    f32): the x-conv P_x = Xh Wh + Xl Wh + Xh Wl runs at bf16 matmul speed
    with ~1e-5 accuracy (bf16 products are exact in fp32 PSUM accumulate).
    input_g stays float32r (fast PE path); its rounding error only enters
    through the sigmoid gate, which attenuates it.
  - Device (per core, 32 blocks x 512 pixels, DMA-grouped by 4 blocks,
    1-stage software pipeline so the PE never stalls on the shared-bank WAR):
      P_x  = double-bf16 conv            (6 matmuls -> PSUM bank)
      xv   = P_x + D_x                   (DVE -> SBUF)
      P_s  = P_x += Wg'^T Xg             (2 f32r matmuls accumulate onto the
                                          same bank after xv is read)
      s    = relu(P_s + D_s)             (ACT, -> float32r SBUF)
      P_p  = wpo^T s   (1 f32r matmul; wpo = Wp*A_p replicated to 128
                        columns => gate broadcast to all partitions)
      psi  = sigmoid(P_p + d_p)          (ACT)
      o    = xv * psi                    (DVE)
  - Output is produced channels-first [128, 16384] per core; host transposes
    back. All DMA rows are >= 4KB contiguous.
"""
import numpy as np

B, H, W, CIN, F = 8, 128, 128, 256, 128
NPIX = H * W          # 16384 pixels per batch/core
NBLK = 512            # pixels per block (one PSUM bank at f32)
NB = NPIX // NBLK     # 32 blocks
NCORES = 8
EPS = 1e-3

_CACHE = {}


def _build_program(npix=NPIX):
    import sys
    if "/opt/trn_rl_repo" not in sys.path:
        sys.path.insert(0, "/opt/trn_rl_repo")
    from contextlib import ExitStack
    import concourse.bacc as bacc
    import concourse.tile as tile
    from concourse import mybir

    f32 = mybir.dt.float32
    f32r = mybir.dt.float32r
    bf16 = mybir.dt.bfloat16
    AF = mybir.ActivationFunctionType

    nc = bacc.Bacc("TRN2", target_bir_lowering=False, debug=False,
                   enable_asserts=False)
    NPIXL = npix
    xg = nc.dram_tensor("xg", [CIN, NPIXL], f32r, kind="ExternalInput").ap()
    xxh = nc.dram_tensor("xxh", [CIN, NPIXL], bf16, kind="ExternalInput").ap()
    xxl = nc.dram_tensor("xxl", [CIN, NPIXL], bf16, kind="ExternalInput").ap()
    wg = nc.dram_tensor("wg", [CIN, F], f32r, kind="ExternalInput").ap()
    wxh = nc.dram_tensor("wxh", [CIN, F], bf16, kind="ExternalInput").ap()
    wxl = nc.dram_tensor("wxl", [CIN, F], bf16, kind="ExternalInput").ap()
    wpo = nc.dram_tensor("wpo", [F, 128], f32r, kind="ExternalInput").ap()
    dvec = nc.dram_tensor("dvec", [128, 3], f32, kind="ExternalInput").ap()
    out_t = nc.dram_tensor("out_t", [F, NPIXL], f32, kind="ExternalOutput").ap()

    with tile.TileContext(nc) as tc:
        with ExitStack() as ctx:
            consts = ctx.enter_context(tc.tile_pool(name="consts", bufs=1))
            io_in = ctx.enter_context(tc.tile_pool(name="io_in", bufs=3))
            work = ctx.enter_context(tc.tile_pool(name="work", bufs=4))
            io_out = ctx.enter_context(tc.tile_pool(name="io_out", bufs=2))
            px_pool = ctx.enter_context(tc.tile_pool(name="px", bufs=4, space="PSUM"))
            pp_pool = ctx.enter_context(tc.tile_pool(name="pp", bufs=3, space="PSUM"))

            wxh0 = consts.tile([128, F], bf16)
            nc.gpsimd.dma_start(out=wxh0, in_=wxh[0:128, :])
            wxh1 = consts.tile([128, F], bf16)
            nc.gpsimd.dma_start(out=wxh1, in_=wxh[128:256, :])
            wxl0 = consts.tile([128, F], bf16)
            nc.gpsimd.dma_start(out=wxl0, in_=wxl[0:128, :])
            wxl1 = consts.tile([128, F], bf16)
            nc.gpsimd.dma_start(out=wxl1, in_=wxl[128:256, :])
            dv = consts.tile([128, 3], f32)
            nc.gpsimd.dma_start(out=dv, in_=dvec)
            wg0 = consts.tile([128, F], f32r)
            nc.gpsimd.dma_start(out=wg0, in_=wg[0:128, :])
            wg1 = consts.tile([128, F], f32r)
            nc.gpsimd.dma_start(out=wg1, in_=wg[128:256, :])
            wpo_sb = consts.tile([F, 128], f32r)
            nc.gpsimd.dma_start(out=wpo_sb, in_=wpo)
            d_s = dv[:, 0:1]
            d_x = dv[:, 1:2]
            d_p = dv[:, 2:3]

            # HAM pre-warm: ~4us of back-to-back tiny matmuls while the first
            # input group is still loading, so the PE clock is at 2.4GHz when
            # real work starts.  Uses wxh0 (first const to land) as dummy data.
            warm_pool = ctx.enter_context(
                tc.tile_pool(name="warm", bufs=1, space="PSUM"))
            ps_warm = warm_pool.tile([128, 128], f32)
            for w in range(64):
                nc.tensor.matmul(ps_warm, wxh0, wxh0, start=True, stop=True)

            NBL = NPIXL // NBLK
            GSIZES = [4] * (NBL // 4)
            assert sum(GSIZES) == NBL

            # One-stage software pipeline across sub-blocks: emit sub-block
            # i's x-matmuls + xv, THEN sub-block i-1's gate tail (g-matmuls,
            # relu, psi, sigmoid, final mul).  The PE static order then
            # interleaves block i's x-matmuls into the window where block
            # i-1's g-matmuls wait on its xv read (WAR on the shared bank),
            # keeping the PE dense (HAM stays warm).
            LAG = 1
            pendq = []  # (i, ps_x, xv, o_slice, xg0_sl, xg1_sl, group_done)

            def finish(p):
                i, ps_x, xv, o_sl, xg0_sl, xg1_sl, group_done = p
                nc.tensor.matmul(ps_x, wg0, xg0_sl, start=False, stop=False,
                                 skip_group_check=True)
                nc.tensor.matmul(ps_x, wg1, xg1_sl, start=False, stop=True,
                                 skip_group_check=True)
                s_sb = work.tile([128, NBLK], f32r, name=f"s_{i}", tag="s")
                nc.scalar.activation(s_sb, ps_x, AF.Relu, bias=d_s, scale=1.0)
                ps_p = pp_pool.tile([128, NBLK], f32, name=f"ps_p_{i}", tag="ps_p")
                nc.tensor.matmul(ps_p, wpo_sb, s_sb, start=True, stop=True)
                psi = work.tile([128, NBLK], f32, name=f"psi_{i}", tag="psi")
                nc.scalar.activation(psi, ps_p, AF.Sigmoid, bias=d_p, scale=1.0)
                nc.vector.tensor_mul(o_sl, xv, psi)
                if group_done is not None:
                    group_done()

            gstart = 0
            for g, GB in enumerate(GSIZES):
                GBLK = GB * NBLK
                gsl = slice(gstart * NBLK, gstart * NBLK + GBLK)
                xxh_b0 = io_in.tile([128, GBLK], bf16, name=f"xxhb0_{g}", tag="xxhb0")
                nc.sync.dma_start(out=xxh_b0, in_=xxh[0:128, gsl])
                xxh_b1 = io_in.tile([128, GBLK], bf16, name=f"xxhb1_{g}", tag="xxhb1")
                nc.sync.dma_start(out=xxh_b1, in_=xxh[128:256, gsl])
                xxl_b0 = io_in.tile([128, GBLK], bf16, name=f"xxlb0_{g}", tag="xxlb0")
                nc.sync.dma_start(out=xxl_b0, in_=xxl[0:128, gsl])
                xxl_b1 = io_in.tile([128, GBLK], bf16, name=f"xxlb1_{g}", tag="xxlb1")
                nc.sync.dma_start(out=xxl_b1, in_=xxl[128:256, gsl])
                xg_b0 = io_in.tile([128, GBLK], f32r, name=f"xgb0_{g}", tag="xgb0")
                nc.sync.dma_start(out=xg_b0, in_=xg[0:128, gsl])
                xg_b1 = io_in.tile([128, GBLK], f32r, name=f"xgb1_{g}", tag="xgb1")
                nc.sync.dma_start(out=xg_b1, in_=xg[128:256, gsl])

                o_big = io_out.tile([128, GBLK], f32, name=f"ob_{g}", tag="ob")

                def group_flush(gsl=gsl, o_big=o_big):
                    nc.sync.dma_start(out=out_t[:, gsl], in_=o_big)

                for j in range(GB):
                    i = gstart + j
                    sl = slice(j * NBLK, (j + 1) * NBLK)

                    # P_x = Wx'^T Xx via double-bf16: Xh Wh + Xl Wh + Xh Wl
                    ps_x = px_pool.tile([128, NBLK], f32, name=f"ps_x_{i}", tag="ps_x")
                    nc.tensor.matmul(ps_x, wxh0, xxh_b0[:, sl], start=True, stop=False)
                    nc.tensor.matmul(ps_x, wxh1, xxh_b1[:, sl], start=False, stop=False)
                    nc.tensor.matmul(ps_x, wxh0, xxl_b0[:, sl], start=False, stop=False)
                    nc.tensor.matmul(ps_x, wxh1, xxl_b1[:, sl], start=False, stop=False)
                    nc.tensor.matmul(ps_x, wxl0, xxh_b0[:, sl], start=False, stop=False)
                    nc.tensor.matmul(ps_x, wxl1, xxh_b1[:, sl], start=False, stop=True)

                    # xv = P_x + D_x -> SBUF (frees the bank value for reuse)
                    xv = work.tile([128, NBLK], f32, name=f"xv_{i}", tag="xv")
                    nc.vector.tensor_scalar_add(xv, ps_x, d_x)

                    if len(pendq) >= LAG:
                        finish(pendq.pop(0))
                    pendq.append((i, ps_x, xv, o_big[:, sl],
                                  xg_b0[:, sl], xg_b1[:, sl],
                                  group_flush if j == GB - 1 else None))

                gstart += GB
            for p_ in pendq:
                finish(p_)

    nc.compile()
    return nc


def _get_program():
    if "nc" not in _CACHE:
        _CACHE["nc"] = _build_program()
    return _CACHE["nc"]


def kernel(input_g, input_x, Wg, bg, gamma_g, beta_g, mean_g, var_g,
           Wx, bx, gamma_x, beta_x, mean_x, var_x,
           Wp, bp, gamma_p, beta_p, mean_p, var_p):
    import sys
    if "/opt/trn_rl_repo" not in sys.path:
        sys.path.insert(0, "/opt/trn_rl_repo")
    from concourse import bass_utils

    # Accept jax/np arrays alike.
    (input_g, input_x, Wg, bg, gamma_g, beta_g, mean_g, var_g,
     Wx, bx, gamma_x, beta_x, mean_x, var_x,
     Wp, bp, gamma_p, beta_p, mean_p, var_p) = (
        np.asarray(a) for a in (
            input_g, input_x, Wg, bg, gamma_g, beta_g, mean_g, var_g,
            Wx, bx, gamma_x, beta_x, mean_x, var_x,
            Wp, bp, gamma_p, beta_p, mean_p, var_p))

    nc = _get_program()

    # Fold BN (+conv bias) into weights/biases on host, in float64 for accuracy.
    f8 = np.float64
    A_g = (gamma_g.astype(f8) / np.sqrt(var_g.astype(f8) + EPS))
    C_g = beta_g.astype(f8) - mean_g.astype(f8) * A_g + bg.astype(f8) * A_g
    A_x = (gamma_x.astype(f8) / np.sqrt(var_x.astype(f8) + EPS))
    C_x = beta_x.astype(f8) - mean_x.astype(f8) * A_x + bx.astype(f8) * A_x
    A_p = (gamma_p.astype(f8) / np.sqrt(var_p.astype(f8) + EPS))[0]
    C_p = (beta_p.astype(f8) - mean_p.astype(f8) * A_p)[0]

    import ml_dtypes
    bf16 = ml_dtypes.bfloat16

    wg_eff = (Wg.astype(f8) * A_g[None, :]).astype(np.float32)
    wx_eff = (Wx.astype(f8) * A_x[None, :]).astype(np.float32)
    wx_hi = wx_eff.astype(bf16)
    wx_lo = (wx_eff - wx_hi.astype(np.float32)).astype(bf16)
    wpo = np.ascontiguousarray(
        np.repeat((Wp[:, 0].astype(f8) * A_p)[:, None], 128, axis=1)
    ).astype(np.float32)
    d_s = (C_g + C_x).astype(np.float32)
    d_x = C_x.astype(np.float32)
    d_p = np.full((128,), A_p * bp.astype(f8)[0] + C_p, dtype=np.float32)
    dvec = np.ascontiguousarray(np.stack([d_s, d_x, d_p], axis=1))

    in_maps = []
    for b in range(NCORES):
        xg_t = np.ascontiguousarray(input_g[b].reshape(NPIX, CIN).T)
        xx_t = np.ascontiguousarray(input_x[b].reshape(NPIX, CIN).T)
        xx_hi = xx_t.astype(bf16)
        xx_lo = (xx_t - xx_hi.astype(np.float32)).astype(bf16)
        in_maps.append(dict(xg=xg_t, xxh=xx_hi, xxl=xx_lo,
                            wg=wg_eff, wxh=wx_hi, wxl=wx_lo,
                            wpo=wpo, dvec=dvec))

    res = bass_utils.run_bass_kernel_spmd(nc, in_maps,
                                          core_ids=list(range(NCORES)))
    _CACHE["last_results"] = res

    out = np.empty((B, H, W, F), np.float32)
    for b in range(NCORES):
        out[b] = res.results[b]["out_t"].T.reshape(H, W, F)
    return out


# revision 34
# speedup vs baseline: 1.4964x; 1.1204x over previous
"""Attention-gate kernel for Trainium2 (8 NeuronCores, batch-parallel).

Computation (per reference):
    g   = BN_g(input_g @ Wg + bg)          # 1x1 conv 256->128 + BN
    x   = BN_x(input_x @ Wx + bx)          # 1x1 conv 256->128 + BN
    s   = relu(g + x)
    psi = sigmoid(BN_p(s @ Wp + bp))       # 128->1 conv + BN + sigmoid
    out = x * psi                          # [8,128,128,128] f32

Strategy (memory-bound target; ~42MB of HBM traffic per core):
  - Shard batch across the 8 cores (core b <- batch b).
  - Host folds BN+bias into the conv weights (W' = W * A, per-output-channel
    scale) and per-channel bias vectors, and pre-transposes the activations
    to channels-first [256, 16384] so the device sees pure contiguous DMA +
    channel-contraction matmuls (no on-device transposes).
  - input_x is uploaded as a double-bf16 pair (hi + lo, same total bytes as
    f32): the x-conv P_x = Xh Wh + Xl Wh + Xh Wl runs at bf16 matmul speed
    with ~1e-5 accuracy (bf16 products are exact in fp32 PSUM accumulate).
    input_g stays float32r (fast PE path); its rounding error only enters
    through the sigmoid gate, which attenuates it.
  - Device (per core, 32 blocks x 512 pixels, DMA-grouped by 4 blocks,
    1-stage software pipeline so the PE never stalls on the shared-bank WAR):
      P_x  = double-bf16 conv            (6 matmuls -> PSUM bank)
      xv   = P_x + D_x                   (DVE -> SBUF)
      P_s  = P_x += Wg'^T Xg             (2 f32r matmuls accumulate onto the
                                          same bank after xv is read)
      s    = relu(P_s + D_s)             (ACT, -> float32r SBUF)
      P_p  = wpo^T s   (1 f32r matmul; wpo = Wp*A_p replicated to 128
                        columns => gate broadcast to all partitions)
      psi  = sigmoid(P_p + d_p)          (ACT)
      o    = xv * psi                    (DVE)
  - Output is produced channels-first [128, 16384] per core; host transposes
    back. All DMA rows are >= 4KB contiguous.
"""
import numpy as np

B, H, W, CIN, F = 8, 128, 128, 256, 128
NPIX = H * W          # 16384 pixels per batch/core
NBLK = 512            # pixels per block (one PSUM bank at f32)
NB = NPIX // NBLK     # 32 blocks
NCORES = 8
EPS = 1e-3

_CACHE = {}


def _build_program(npix=NPIX):
    import sys
    if "/opt/trn_rl_repo" not in sys.path:
        sys.path.insert(0, "/opt/trn_rl_repo")
    from contextlib import ExitStack
    import concourse.bacc as bacc
    import concourse.tile as tile
    from concourse import mybir

    f32 = mybir.dt.float32
    f32r = mybir.dt.float32r
    bf16 = mybir.dt.bfloat16
    AF = mybir.ActivationFunctionType

    nc = bacc.Bacc("TRN2", target_bir_lowering=False, debug=False,
                   enable_asserts=False)
    NPIXL = npix
    xg = nc.dram_tensor("xg", [CIN, NPIXL], f32r, kind="ExternalInput").ap()
    xxh = nc.dram_tensor("xxh", [CIN, NPIXL], bf16, kind="ExternalInput").ap()
    xxl = nc.dram_tensor("xxl", [CIN, NPIXL], bf16, kind="ExternalInput").ap()
    wg = nc.dram_tensor("wg", [CIN, F], f32r, kind="ExternalInput").ap()
    wxh = nc.dram_tensor("wxh", [CIN, F], bf16, kind="ExternalInput").ap()
    wxl = nc.dram_tensor("wxl", [CIN, F], bf16, kind="ExternalInput").ap()
    wpo = nc.dram_tensor("wpo", [F, 128], f32r, kind="ExternalInput").ap()
    dvec = nc.dram_tensor("dvec", [128, 3], f32, kind="ExternalInput").ap()
    out_t = nc.dram_tensor("out_t", [F, NPIXL], f32, kind="ExternalOutput").ap()

    with tile.TileContext(nc) as tc:
        with ExitStack() as ctx:
            consts = ctx.enter_context(tc.tile_pool(name="consts", bufs=1))
            io_in = ctx.enter_context(tc.tile_pool(name="io_in", bufs=3))
            work = ctx.enter_context(tc.tile_pool(name="work", bufs=4))
            io_out = ctx.enter_context(tc.tile_pool(name="io_out", bufs=2))
            px_pool = ctx.enter_context(tc.tile_pool(name="px", bufs=4, space="PSUM"))
            pp_pool = ctx.enter_context(tc.tile_pool(name="pp", bufs=3, space="PSUM"))

            wxh0 = consts.tile([128, F], bf16)
            nc.gpsimd.dma_start(out=wxh0, in_=wxh[0:128, :])
            wxh1 = consts.tile([128, F], bf16)
            nc.gpsimd.dma_start(out=wxh1, in_=wxh[128:256, :])
            wxl0 = consts.tile([128, F], bf16)
            nc.gpsimd.dma_start(out=wxl0, in_=wxl[0:128, :])
            wxl1 = consts.tile([128, F], bf16)
            nc.gpsimd.dma_start(out=wxl1, in_=wxl[128:256, :])
            dv = consts.tile([128, 3], f32)
            nc.gpsimd.dma_start(out=dv, in_=dvec)
            wg0 = consts.tile([128, F], f32r)
            nc.gpsimd.dma_start(out=wg0, in_=wg[0:128, :])
            wg1 = consts.tile([128, F], f32r)
            nc.gpsimd.dma_start(out=wg1, in_=wg[128:256, :])
            wpo_sb = consts.tile([F, 128], f32r)
            nc.gpsimd.dma_start(out=wpo_sb, in_=wpo)
            d_s = dv[:, 0:1]
            d_x = dv[:, 1:2]
            d_p = dv[:, 2:3]

            NBL = NPIXL // NBLK
            GSIZES = [4] * (NBL // 4)
            assert sum(GSIZES) == NBL

            # One-stage software pipeline across sub-blocks: emit sub-block
            # i's x-matmuls + xv, THEN sub-block i-1's gate tail (g-matmuls,
            # relu, psi, sigmoid, final mul).  The PE static order then
            # interleaves block i's x-matmuls into the window where block
            # i-1's g-matmuls wait on its xv read (WAR on the shared bank),
            # keeping the PE dense (HAM stays warm).
            LAG = 1
            pendq = []  # (i, ps_x, xv, o_slice, xg0_sl, xg1_sl, group_done)

            def finish(p):
                i, ps_x, xv, o_sl, xg0_sl, xg1_sl, group_done = p
                nc.tensor.matmul(ps_x, wg0, xg0_sl, start=False, stop=False,
                                 skip_group_check=True)
                nc.tensor.matmul(ps_x, wg1, xg1_sl, start=False, stop=True,
                                 skip_group_check=True)
                s_sb = work.tile([128, NBLK], f32r, name=f"s_{i}", tag="s")
                nc.scalar.activation(s_sb, ps_x, AF.Relu, bias=d_s, scale=1.0)
                ps_p = pp_pool.tile([128, NBLK], f32, name=f"ps_p_{i}", tag="ps_p")
                nc.tensor.matmul(ps_p, wpo_sb, s_sb, start=True, stop=True)
                psi = work.tile([128, NBLK], f32, name=f"psi_{i}", tag="psi")
                nc.scalar.activation(psi, ps_p, AF.Sigmoid, bias=d_p, scale=1.0)
                nc.vector.tensor_mul(o_sl, xv, psi)
                if group_done is not None:
                    group_done()

            gstart = 0
            for g, GB in enumerate(GSIZES):
                GBLK = GB * NBLK
                gsl = slice(gstart * NBLK, gstart * NBLK + GBLK)
                xxh_b0 = io_in.tile([128, GBLK], bf16, name=f"xxhb0_{g}", tag="xxhb0")
                nc.sync.dma_start(out=xxh_b0, in_=xxh[0:128, gsl])
                xxh_b1 = io_in.tile([128, GBLK], bf16, name=f"xxhb1_{g}", tag="xxhb1")
                nc.sync.dma_start(out=xxh_b1, in_=xxh[128:256, gsl])
                xxl_b0 = io_in.tile([128, GBLK], bf16, name=f"xxlb0_{g}", tag="xxlb0")
                nc.sync.dma_start(out=xxl_b0, in_=xxl[0:128, gsl])
                xxl_b1 = io_in.tile([128, GBLK], bf16, name=f"xxlb1_{g}", tag="xxlb1")
                nc.sync.dma_start(out=xxl_b1, in_=xxl[128:256, gsl])
                xg_b0 = io_in.tile([128, GBLK], f32r, name=f"xgb0_{g}", tag="xgb0")
                nc.sync.dma_start(out=xg_b0, in_=xg[0:128, gsl])
                xg_b1 = io_in.tile([128, GBLK], f32r, name=f"xgb1_{g}", tag="xgb1")
                nc.sync.dma_start(out=xg_b1, in_=xg[128:256, gsl])

                o_big = io_out.tile([128, GBLK], f32, name=f"ob_{g}", tag="ob")

                def group_flush(gsl=gsl, o_big=o_big):
                    nc.sync.dma_start(out=out_t[:, gsl], in_=o_big)

                for j in range(GB):
                    i = gstart + j
                    sl = slice(j * NBLK, (j + 1) * NBLK)

                    # P_x = Wx'^T Xx via double-bf16: Xh Wh + Xl Wh + Xh Wl
                    ps_x = px_pool.tile([128, NBLK], f32, name=f"ps_x_{i}", tag="ps_x")
                    nc.tensor.matmul(ps_x, wxh0, xxh_b0[:, sl], start=True, stop=False)
                    nc.tensor.matmul(ps_x, wxh1, xxh_b1[:, sl], start=False, stop=False)
                    nc.tensor.matmul(ps_x, wxh0, xxl_b0[:, sl], start=False, stop=False)
                    nc.tensor.matmul(ps_x, wxh1, xxl_b1[:, sl], start=False, stop=False)
                    nc.tensor.matmul(ps_x, wxl0, xxh_b0[:, sl], start=False, stop=False)
                    nc.tensor.matmul(ps_x, wxl1, xxh_b1[:, sl], start=False, stop=True)

                    # xv = P_x + D_x -> SBUF (frees the bank value for reuse)
                    xv = work.tile([128, NBLK], f32, name=f"xv_{i}", tag="xv")
                    nc.vector.tensor_scalar_add(xv, ps_x, d_x)

                    if len(pendq) >= LAG:
                        finish(pendq.pop(0))
                    pendq.append((i, ps_x, xv, o_big[:, sl],
                                  xg_b0[:, sl], xg_b1[:, sl],
                                  group_flush if j == GB - 1 else None))

                gstart += GB
            for p_ in pendq:
                finish(p_)

    nc.compile()
    return nc


def _get_program():
    if "nc" not in _CACHE:
        _CACHE["nc"] = _build_program()
    return _CACHE["nc"]


def kernel(input_g, input_x, Wg, bg, gamma_g, beta_g, mean_g, var_g,
           Wx, bx, gamma_x, beta_x, mean_x, var_x,
           Wp, bp, gamma_p, beta_p, mean_p, var_p):
    import sys
    if "/opt/trn_rl_repo" not in sys.path:
        sys.path.insert(0, "/opt/trn_rl_repo")
    from concourse import bass_utils

    # Accept jax/np arrays alike.
    (input_g, input_x, Wg, bg, gamma_g, beta_g, mean_g, var_g,
     Wx, bx, gamma_x, beta_x, mean_x, var_x,
     Wp, bp, gamma_p, beta_p, mean_p, var_p) = (
        np.asarray(a) for a in (
            input_g, input_x, Wg, bg, gamma_g, beta_g, mean_g, var_g,
            Wx, bx, gamma_x, beta_x, mean_x, var_x,
            Wp, bp, gamma_p, beta_p, mean_p, var_p))

    nc = _get_program()

    # Fold BN (+conv bias) into weights/biases on host, in float64 for accuracy.
    f8 = np.float64
    A_g = (gamma_g.astype(f8) / np.sqrt(var_g.astype(f8) + EPS))
    C_g = beta_g.astype(f8) - mean_g.astype(f8) * A_g + bg.astype(f8) * A_g
    A_x = (gamma_x.astype(f8) / np.sqrt(var_x.astype(f8) + EPS))
    C_x = beta_x.astype(f8) - mean_x.astype(f8) * A_x + bx.astype(f8) * A_x
    A_p = (gamma_p.astype(f8) / np.sqrt(var_p.astype(f8) + EPS))[0]
    C_p = (beta_p.astype(f8) - mean_p.astype(f8) * A_p)[0]

    import ml_dtypes
    bf16 = ml_dtypes.bfloat16

    wg_eff = (Wg.astype(f8) * A_g[None, :]).astype(np.float32)
    wx_eff = (Wx.astype(f8) * A_x[None, :]).astype(np.float32)
    wx_hi = wx_eff.astype(bf16)
    wx_lo = (wx_eff - wx_hi.astype(np.float32)).astype(bf16)
    wpo = np.ascontiguousarray(
        np.repeat((Wp[:, 0].astype(f8) * A_p)[:, None], 128, axis=1)
    ).astype(np.float32)
    d_s = (C_g + C_x).astype(np.float32)
    d_x = C_x.astype(np.float32)
    d_p = np.full((128,), A_p * bp.astype(f8)[0] + C_p, dtype=np.float32)
    dvec = np.ascontiguousarray(np.stack([d_s, d_x, d_p], axis=1))

    in_maps = []
    for b in range(NCORES):
        xg_t = np.ascontiguousarray(input_g[b].reshape(NPIX, CIN).T)
        xx_t = np.ascontiguousarray(input_x[b].reshape(NPIX, CIN).T)
        xx_hi = xx_t.astype(bf16)
        xx_lo = (xx_t - xx_hi.astype(np.float32)).astype(bf16)
        in_maps.append(dict(xg=xg_t, xxh=xx_hi, xxl=xx_lo,
                            wg=wg_eff, wxh=wx_hi, wxl=wx_lo,
                            wpo=wpo, dvec=dvec))

    res = bass_utils.run_bass_kernel_spmd(nc, in_maps,
                                          core_ids=list(range(NCORES)))
    _CACHE["last_results"] = res

    out = np.empty((B, H, W, F), np.float32)
    for b in range(NCORES):
        out[b] = res.results[b]["out_t"].T.reshape(H, W, F)
    return out
